# revision 9
# baseline (speedup 1.0000x reference)
"""Trainium2 Bass kernel for nn_MoCWrapper (topk_masking).

Reference semantics per layer l (L=16, B=1, T=2048, D=2048, R=128, K=1024):
  h            = relu(hidden @ W_r1 + b_r1)            # (T, R)
  logits       = h @ W_r2 + b_r2                       # (T,)
  probs        = sigmoid(logits)
  s2_mask      = one-hot top-K(probs) over tokens      # (T,)
  s1           = hidden @ W_s1 + b_s1                  # (T, D)
  m            = (s2_mask - probs) + probs
  out          = m * s2 + (1 - m) * s1

Sharding: layer dim L=16 across 8 cores (2 layers/core), SPMD one NEFF.

Per-core schedule per layer:
  phase T+R (fused): stream hidden tiles, PE-transpose each 128x128 block to
    PSUM, copy it twice: (a) DVE -> small fp32 streaming buffer consumed
    immediately by the fp32 router matmul (top-k selection must reproduce the
    reference ordering; boundary gaps are ~1e-5 in prob space so reduced
    precision is NOT safe here), (b) ACT -> resident hT tile declared float32r
    (rounds; these are hT's only writers, which the walrus fp32r verifier
    requires). Router logits per 128-token chunk -> row scratch (DRAM) +
    column layout [128,16] via tiny PE transposes of the same values.
  phase M: rank_i = #{j: logit_j > logit_i} via DVE tensor_scalar(is_gt) with
    fused accumulate; mask = rank < K. Equals lax.top_k selection when the
    K-th value is unique (holds for this data; verified in test).
  phase S: s1 = hT.T @ Ws1 in float32r (full PE rate at free-dim >= 256),
    Ws1 streamed in 256-wide f32r chunks; blend fused on PSUM with DVE.
"""

import numpy as np

L_TOT, B, T, D, R = 16, 1, 2048, 2048, 128
KSEL = 1024
N_CORES = 8
L_PER = L_TOT // N_CORES  # 2

TK = D // 128  # 16 contraction tiles
TT = T // 128  # 16 token chunks
EC = 256       # e-chunk width for s1/blend
NEC = D // EC  # 8

# how Ws1 gets rounded to f32r: "cast_dma" (gpsimd DMA casts on load) or
# "dve" (load raw f32 halves, DVE-rounds into the f32r tile)
WS1_MODE = "cast_dma"

_built = None


def _build():
    global _built
    if _built is not None:
        return _built

    import concourse.mybir as mybir
    import concourse.tile as tile
    from concourse import bacc
    from concourse.masks import make_identity

    f32 = mybir.dt.float32
    f32r = mybir.dt.float32r
    Alu = mybir.AluOpType
    Act = mybir.ActivationFunctionType
    X = mybir.AxisListType.X

    nc = bacc.Bacc("TRN2", target_bir_lowering=False, debug=False)

    hidden = nc.dram_tensor("hidden", [L_PER, T, D], f32, kind="ExternalInput").ap()
    s2d = nc.dram_tensor("s2", [L_PER, T, D], f32, kind="ExternalInput").ap()
    wr1d = nc.dram_tensor("w_r1", [L_PER, D, R], f32, kind="ExternalInput").ap()
    br1d = nc.dram_tensor("b_r1", [L_PER, R], f32, kind="ExternalInput").ap()
    wr2d = nc.dram_tensor("w_r2", [L_PER, R], f32, kind="ExternalInput").ap()
    br2d = nc.dram_tensor("b_r2", [L_PER], f32, kind="ExternalInput").ap()
    ws1d = nc.dram_tensor("w_s1", [L_PER, D, D], f32, kind="ExternalInput").ap()
    bs1d = nc.dram_tensor("b_s1", [L_PER, D], f32, kind="ExternalInput").ap()
    outd = nc.dram_tensor("out", [L_PER, T, D], f32, kind="ExternalOutput").ap()

    from contextlib import ExitStack

    with ExitStack() as ctx:
        tc = ctx.enter_context(tile.TileContext(nc))
        constp = ctx.enter_context(tc.tile_pool(name="constp", bufs=1))
        hTp = ctx.enter_context(tc.tile_pool(name="hTp", bufs=1))
        stgp = ctx.enter_context(tc.tile_pool(name="stgp", bufs=2))
        strmp = ctx.enter_context(tc.tile_pool(name="strmp", bufs=4))
        wtsp = ctx.enter_context(tc.tile_pool(name="wtsp", bufs=1))
        rowp = ctx.enter_context(tc.tile_pool(name="rowp", bufs=2))
        lbp = ctx.enter_context(tc.tile_pool(name="lbp", bufs=1))
        ws1p = ctx.enter_context(tc.tile_pool(name="ws1p", bufs=2))
        blendp = ctx.enter_context(tc.tile_pool(name="blendp", bufs=2))
        smallp = ctx.enter_context(tc.tile_pool(name="smallp", bufs=1))
        dramp = ctx.enter_context(tc.tile_pool(name="dramp", bufs=1, space="DRAM"))
        psTp = ctx.enter_context(tc.tile_pool(name="psTp", bufs=2, space="PSUM"))
        psRp = ctx.enter_context(tc.tile_pool(name="psRp", bufs=1, space="PSUM"))
        psLRp = ctx.enter_context(tc.tile_pool(name="psLRp", bufs=1, space="PSUM"))
        psLCp = ctx.enter_context(tc.tile_pool(name="psLCp", bufs=1, space="PSUM"))
        psSp = ctx.enter_context(tc.tile_pool(name="psSp", bufs=3, space="PSUM"))
        ws1rawp = ws1p if WS1_MODE == "cast_dma" else ctx.enter_context(
            tc.tile_pool(name="ws1rawp", bufs=2)
        )

        identity = constp.tile([128, 128], f32)
        make_identity(nc, identity)

        for l in range(L_PER):
            # --- per-layer weight loads -------------------------------------
            wr1 = wtsp.tile([128, TK, R], f32, tag="wr1")
            nc.sync.dma_start(wr1, wr1d[l].rearrange("(k p) r -> p k r", p=128))
            wr2 = smallp.tile([128, 1], f32, tag="wr2")
            nc.sync.dma_start(wr2, wr2d[l].unsqueeze(1))
            br1 = smallp.tile([128, 1], f32, tag="br1")
            nc.sync.dma_start(br1, br1d[l].unsqueeze(1))
            br2c = smallp.tile([128, 1], f32, tag="br2c")
            nc.sync.dma_start(br2c, br2d[l : l + 1].unsqueeze(0).to_broadcast((128, 1)))

            # --- fused transpose + router over t-chunks ----------------------
            hT = hTp.tile([128, TK, T], f32r, tag="hT")
            scr = dramp.tile([1, T], f32, tag="scr")
            psLC = psLCp.tile([128, 16], f32, tag="psLC")
            for t in range(TT):
                psR = psRp.tile([128, 128], f32, tag="psR")
                strms = []
                for half in range(2):
                    stg = stgp.tile([128, 1024], f32, tag="stg")
                    nc.sync.dma_start(
                        stg,
                        hidden[l, t * 128 : (t + 1) * 128,
                               half * 1024 : (half + 1) * 1024],
                    )
                    for g in range(2):
                        psT = psTp.tile([128, 512], f32, tag="psT")
                        for i in range(4):
                            nc.tensor.transpose(
                                psT[:, i * 128 : (i + 1) * 128],
                                stg[:, (g * 4 + i) * 128 : (g * 4 + i + 1) * 128],
                                identity,
                            )
                        k0 = half * 8 + g * 4
                        psT3 = psT.rearrange("p (a b) -> p a b", a=4)
                        strm = strmp.tile([128, 4, 128], f32, tag="strm")
                        nc.vector.tensor_copy(strm, psT3)
                        nc.scalar.activation(
                            hT[:, k0 : k0 + 4, t * 128 : (t + 1) * 128],
                            psT3,
                            Act.Copy,
                        )
                        strms.append((k0, strm))
                # dense fp32 router matmul for this t-chunk (N=128)
                for k0, strm in strms:
                    for i in range(4):
                        nc.tensor.matmul(
                            psR,
                            wr1[:, k0 + i, :],
                            strm[:, i, :],
                            start=(k0 + i == 0),
                            stop=(k0 + i == TK - 1),
                        )
                hrelu = rowp.tile([128, 128], f32, tag="hrelu")
                nc.scalar.activation(hrelu, psR, Act.Relu, bias=br1)
                psLR = psLRp.tile([1, 128], f32, tag="psLR")
                nc.tensor.matmul(psLR, wr2, hrelu, start=True, stop=True)
                row_sb = rowp.tile([1, 128], f32, tag="rowsb")
                nc.scalar.activation(row_sb, psLR, Act.Copy)
                nc.scalar.dma_start(scr[0:1, t * 128 : (t + 1) * 128], row_sb)
                # column layout of the same logit values (bitwise identical)
                nc.tensor.transpose(
                    psLC[:, t : t + 1], row_sb, identity[0:1, 0:1]
                )

            # --- phase M: rank -> mask -> blend coefficients -----------------
            lb = lbp.tile([128, T], f32, tag="lb")
            nc.sync.dma_start(lb, scr[0:1, :].to_broadcast((128, T)))
            rank4 = smallp.tile([128, 16, 4], f32, tag="rank4")
            for c in range(16):
                for q in range(4):
                    cmp = stgp.tile([128, 512], f32, tag="stg", name="cmp")
                    nc.vector.tensor_scalar(
                        cmp,
                        lb[:, q * 512 : (q + 1) * 512],
                        psLC[:, c : c + 1],
                        None,
                        op0=Alu.is_gt,
                        op1=Alu.add,
                        accum_out=rank4[:, c, q : q + 1],
                    )
            rank = smallp.tile([128, 16], f32, tag="rank")
            nc.vector.tensor_reduce(rank, rank4, axis=X, op=Alu.add)
            probsC = smallp.tile([128, 16], f32, tag="probsC")
            nc.scalar.activation(probsC, psLC, Act.Sigmoid, bias=br2c)
            s2m = smallp.tile([128, 16], f32, tag="s2m")
            nc.vector.tensor_scalar(s2m, rank, float(KSEL), None, op0=Alu.is_lt)
            m1 = smallp.tile([128, 16], f32, tag="m1")
            nc.vector.tensor_sub(m1, s2m, probsC)
            mst = smallp.tile([128, 16], f32, tag="mst")
            nc.vector.tensor_add(mst, m1, probsC)
            onem = smallp.tile([128, 16], f32, tag="onem")
            nc.vector.tensor_scalar(onem, mst, -1.0, 1.0, op0=Alu.mult, op1=Alu.add)

            # --- phase S: s1 matmul (f32r) + blend ---------------------------
            ws1r3 = ws1d[l].rearrange("(k p) e -> p k e", p=128)
            for ec in range(NEC):
                ws1 = ws1p.tile([128, TK, EC], f32r, tag="ws1")
                if WS1_MODE == "cast_dma":
                    nc.gpsimd.dma_start(
                        ws1, ws1r3[:, :, ec * EC : (ec + 1) * EC]
                    )
                else:
                    for hf in range(2):
                        e0 = ec * EC + hf * (EC // 2)
                        ws1raw = ws1rawp.tile(
                            [128, TK, EC // 2], f32, tag="ws1raw"
                        )
                        nc.sync.dma_start(
                            ws1raw, ws1r3[:, :, e0 : e0 + EC // 2]
                        )
                        nc.vector.tensor_copy(
                            ws1[:, :, hf * (EC // 2) : (hf + 1) * (EC // 2)],
                            ws1raw,
                        )
                biast = blendp.tile([128, EC], f32, tag="bias")
                nc.sync.dma_start(
                    biast,
                    bs1d[l, ec * EC : (ec + 1) * EC]
                    .unsqueeze(0)
                    .to_broadcast((128, EC)),
                )
                for t in range(TT):
                    ps = psSp.tile([128, EC], f32, tag="psS")
                    for k in range(TK):
                        nc.tensor.matmul(
                            ps,
                            hT[:, k, t * 128 : (t + 1) * 128],
                            ws1[:, k, :],
                            start=(k == 0),
                            stop=(k == TK - 1),
                        )
                    s2t = blendp.tile([128, EC], f32, tag="s2t")
                    nc.sync.dma_start(
                        s2t,
                        s2d[l, t * 128 : (t + 1) * 128, ec * EC : (ec + 1) * EC],
                    )
                    tmp = blendp.tile([128, EC], f32, tag="tmp")
                    nc.vector.tensor_scalar(
                        tmp, s2t, mst[:, t : t + 1], None, op0=Alu.mult
                    )
                    s1b = blendp.tile([128, EC], f32, tag="s1b")
                    nc.vector.tensor_add(s1b, ps, biast)
                    outt = blendp.tile([128, EC], f32, tag="s2t", name="outt")
                    nc.vector.scalar_tensor_tensor(
                        outt,
                        in0=s1b,
                        scalar=onem[:, t : t + 1],
                        in1=tmp,
                        op0=Alu.mult,
                        op1=Alu.add,
                    )
                    nc.scalar.dma_start(
                        outd[l, t * 128 : (t + 1) * 128, ec * EC : (ec + 1) * EC],
                        outt,
                    )

    nc.compile()
    _built = nc
    return _built


def kernel(**inputs):
    nc = _build()
    from concourse.bass_utils import run_bass_kernel_spmd

    hid = np.ascontiguousarray(
        np.asarray(inputs["hidden"], dtype=np.float32).reshape(L_TOT, T, D)
    )
    s2 = np.ascontiguousarray(
        np.asarray(inputs["s2"], dtype=np.float32).reshape(L_TOT, T, D)
    )
    wr1 = np.asarray(inputs["W_r1"], dtype=np.float32)
    br1 = np.asarray(inputs["b_r1"], dtype=np.float32)
    wr2 = np.asarray(inputs["W_r2"], dtype=np.float32)
    br2 = np.asarray(inputs["b_r2"], dtype=np.float32)
    ws1 = np.asarray(inputs["W_s1"], dtype=np.float32)
    bs1 = np.asarray(inputs["b_s1"], dtype=np.float32)

    in_maps = []
    for c in range(N_CORES):
        sl = slice(c * L_PER, (c + 1) * L_PER)
        in_maps.append(
            {
                "hidden": np.ascontiguousarray(hid[sl]),
                "s2": np.ascontiguousarray(s2[sl]),
                "w_r1": np.ascontiguousarray(wr1[sl]),
                "b_r1": np.ascontiguousarray(br1[sl]),
                "w_r2": np.ascontiguousarray(wr2[sl]),
                "b_r2": np.ascontiguousarray(br2[sl]),
                "w_s1": np.ascontiguousarray(ws1[sl]),
                "b_s1": np.ascontiguousarray(bs1[sl]),
            }
        )

    res = run_bass_kernel_spmd(nc, in_maps, core_ids=list(range(N_CORES)))
    out = np.concatenate(
        [r["out"].reshape(L_PER, B, T, D) for r in res.results], axis=0
    )
    return np.ascontiguousarray(out.astype(np.float32))


if __name__ == "__main__":
    _build()
    print("build OK")


# revision 11
# speedup vs baseline: 52422.5555x; 52422.5555x over previous
"""Trainium2 Bass kernel for nn_MoCWrapper (topk_masking).

Reference semantics per layer l (L=16, B=1, T=2048, D=2048, R=128, K=1024):
  h            = relu(hidden @ W_r1 + b_r1)            # (T, R)
  logits       = h @ W_r2 + b_r2                       # (T,)
  probs        = sigmoid(logits)
  s2_mask      = one-hot top-K(probs) over tokens      # (T,)
  s1           = hidden @ W_s1 + b_s1                  # (T, D)
  m            = (s2_mask - probs) + probs
  out          = m * s2 + (1 - m) * s1

Sharding: layer dim L=16 across 8 cores (2 layers/core), SPMD one NEFF.

Per-core schedule per layer:
  phase T+R (fused): stream hidden tiles, PE-transpose each 128x128 block to
    PSUM, copy it twice: (a) DVE -> small fp32 streaming buffer consumed
    immediately by the fp32 router matmul (top-k selection must reproduce the
    reference ordering; boundary gaps are ~1e-5 in prob space so reduced
    precision is NOT safe here), (b) ACT -> resident hT tile declared float32r
    (rounds; these are hT's only writers, which the walrus fp32r verifier
    requires). Router logits per 128-token chunk -> row scratch (DRAM) +
    column layout [128,16] via tiny PE transposes of the same values.
  phase M: rank_i = #{j: logit_j > logit_i} via DVE tensor_scalar(is_gt) with
    fused accumulate; mask = rank < K. Equals lax.top_k selection when the
    K-th value is unique (holds for this data; verified in test).
  phase S: s1 = hT.T @ Ws1 in float32r (full PE rate at free-dim >= 256),
    Ws1 streamed in 256-wide f32r chunks; blend fused on PSUM with DVE.
"""

import numpy as np

L_TOT, B, T, D, R = 16, 1, 2048, 2048, 128
KSEL = 1024
N_CORES = 8
L_PER = L_TOT // N_CORES  # 2

TK = D // 128  # 16 contraction tiles
TT = T // 128  # 16 token chunks
EC = 256       # e-chunk width for s1/blend
NEC = D // EC  # 8

# how Ws1 gets rounded to f32r: "cast_dma" (gpsimd DMA casts on load) or
# "dve" (load raw f32 halves, DVE-rounds into the f32r tile)
WS1_MODE = "cast_dma"

_built = None


def _build():
    global _built
    if _built is not None:
        return _built

    import concourse.mybir as mybir
    import concourse.tile as tile
    from concourse import bacc
    from concourse.masks import make_identity

    f32 = mybir.dt.float32
    f32r = mybir.dt.float32r
    Alu = mybir.AluOpType
    Act = mybir.ActivationFunctionType
    X = mybir.AxisListType.X

    nc = bacc.Bacc("TRN2", target_bir_lowering=False, debug=False)

    hidden = nc.dram_tensor("hidden", [L_PER, T, D], f32, kind="ExternalInput").ap()
    s2d = nc.dram_tensor("s2", [L_PER, T, D], f32, kind="ExternalInput").ap()
    wr1d = nc.dram_tensor("w_r1", [L_PER, D, R], f32, kind="ExternalInput").ap()
    br1d = nc.dram_tensor("b_r1", [L_PER, R], f32, kind="ExternalInput").ap()
    wr2d = nc.dram_tensor("w_r2", [L_PER, R], f32, kind="ExternalInput").ap()
    br2d = nc.dram_tensor("b_r2", [L_PER], f32, kind="ExternalInput").ap()
    ws1d = nc.dram_tensor("w_s1", [L_PER, D, D], f32, kind="ExternalInput").ap()
    bs1d = nc.dram_tensor("b_s1", [L_PER, D], f32, kind="ExternalInput").ap()
    outd = nc.dram_tensor("out", [L_PER, T, D], f32, kind="ExternalOutput").ap()

    from contextlib import ExitStack

    with ExitStack() as ctx:
        tc = ctx.enter_context(tile.TileContext(nc))
        constp = ctx.enter_context(tc.tile_pool(name="constp", bufs=1))
        hTp = ctx.enter_context(tc.tile_pool(name="hTp", bufs=1))
        stgp = ctx.enter_context(tc.tile_pool(name="stgp", bufs=2))
        strmp = ctx.enter_context(tc.tile_pool(name="strmp", bufs=4))
        wtsp = ctx.enter_context(tc.tile_pool(name="wtsp", bufs=1))
        rowp = ctx.enter_context(tc.tile_pool(name="rowp", bufs=2))
        lbp = ctx.enter_context(tc.tile_pool(name="lbp", bufs=1))
        ws1p = ctx.enter_context(tc.tile_pool(name="ws1p", bufs=2))
        blendp = ctx.enter_context(tc.tile_pool(name="blendp", bufs=2))
        smallp = ctx.enter_context(tc.tile_pool(name="smallp", bufs=1))
        dramp = ctx.enter_context(tc.tile_pool(name="dramp", bufs=1, space="DRAM"))
        psTp = ctx.enter_context(tc.tile_pool(name="psTp", bufs=2, space="PSUM"))
        psRp = ctx.enter_context(tc.tile_pool(name="psRp", bufs=1, space="PSUM"))
        psLCp = ctx.enter_context(tc.tile_pool(name="psLCp", bufs=1, space="PSUM"))
        psSp = ctx.enter_context(tc.tile_pool(name="psSp", bufs=4, space="PSUM"))
        ws1rawp = ws1p if WS1_MODE == "cast_dma" else ctx.enter_context(
            tc.tile_pool(name="ws1rawp", bufs=2)
        )

        identity = constp.tile([128, 128], f32)
        make_identity(nc, identity)

        for l in range(L_PER):
            # --- per-layer weight loads -------------------------------------
            wr1 = wtsp.tile([128, TK, R], f32, tag="wr1")
            nc.sync.dma_start(wr1, wr1d[l].rearrange("(k p) r -> p k r", p=128))
            wr2 = smallp.tile([128, 1], f32, tag="wr2")
            nc.sync.dma_start(wr2, wr2d[l].unsqueeze(1))
            br1 = smallp.tile([128, 1], f32, tag="br1")
            nc.sync.dma_start(br1, br1d[l].unsqueeze(1))
            br2c = smallp.tile([128, 1], f32, tag="br2c")
            nc.sync.dma_start(br2c, br2d[l : l + 1].unsqueeze(0).to_broadcast((128, 1)))

            # --- fused transpose + router over t-chunks ----------------------
            hT = hTp.tile([128, TK, T], f32r, tag="hT")
            scr = dramp.tile([1, T], f32, tag="scr")
            psLC = psLCp.tile([128, 16], f32, tag="psLC")
            for t in range(TT):
                psR = psRp.tile([128, 128], f32, tag="psR")
                strms = []
                for half in range(2):
                    stg = stgp.tile([128, 1024], f32, tag="stg")
                    nc.sync.dma_start(
                        stg,
                        hidden[l, t * 128 : (t + 1) * 128,
                               half * 1024 : (half + 1) * 1024],
                    )
                    for g in range(2):
                        psT = psTp.tile([128, 512], f32, tag="psT")
                        for i in range(4):
                            nc.tensor.transpose(
                                psT[:, i * 128 : (i + 1) * 128],
                                stg[:, (g * 4 + i) * 128 : (g * 4 + i + 1) * 128],
                                identity,
                            )
                        k0 = half * 8 + g * 4
                        psT3 = psT.rearrange("p (a b) -> p a b", a=4)
                        strm = strmp.tile([128, 4, 128], f32, tag="strm")
                        nc.vector.tensor_copy(strm, psT3)
                        nc.scalar.activation(
                            hT[:, k0 : k0 + 4, t * 128 : (t + 1) * 128],
                            psT3,
                            Act.Copy,
                        )
                        strms.append((k0, strm))
                # dense fp32 router matmul for this t-chunk (N=128)
                for k0, strm in strms:
                    for i in range(4):
                        nc.tensor.matmul(
                            psR,
                            wr1[:, k0 + i, :],
                            strm[:, i, :],
                            start=(k0 + i == 0),
                            stop=(k0 + i == TK - 1),
                        )
                hrelu = rowp.tile([128, 128], f32, tag="hrelu")
                nc.scalar.activation(hrelu, psR, Act.Relu, bias=br1)
                # logits directly in column layout: out[128t, 1] at column t
                nc.tensor.matmul(
                    psLC[:, t : t + 1], hrelu, wr2, start=True, stop=True
                )

            # row layout derives from the same psLC values (bit-exact copies):
            # colsb -> DRAM scratch (strided so scr[t] = logit_t) -> broadcast
            colsb = smallp.tile([128, 16], f32, tag="colsb")
            nc.scalar.activation(colsb, psLC, Act.Copy)
            nc.scalar.dma_start(
                scr[0:1, :].rearrange("a (c p) -> (a p) c", p=128), colsb
            )

            # --- phase M: rank -> mask -> blend coefficients -----------------
            lb = lbp.tile([128, T], f32, tag="lb")
            nc.sync.dma_start(lb, scr[0:1, :].to_broadcast((128, T)))
            rank4 = smallp.tile([128, 16, 4], f32, tag="rank4")
            for c in range(16):
                for q in range(4):
                    cmp = stgp.tile([128, 512], f32, tag="stg", name="cmp")
                    nc.vector.tensor_scalar(
                        cmp,
                        lb[:, q * 512 : (q + 1) * 512],
                        psLC[:, c : c + 1],
                        None,
                        op0=Alu.is_gt,
                        op1=Alu.add,
                        accum_out=rank4[:, c, q : q + 1],
                    )
            rank = smallp.tile([128, 16], f32, tag="rank")
            nc.vector.tensor_reduce(rank, rank4, axis=X, op=Alu.add)
            probsC = smallp.tile([128, 16], f32, tag="probsC")
            nc.scalar.activation(probsC, psLC, Act.Sigmoid, bias=br2c)
            s2m = smallp.tile([128, 16], f32, tag="s2m")
            nc.vector.tensor_scalar(s2m, rank, float(KSEL), None, op0=Alu.is_lt)
            m1 = smallp.tile([128, 16], f32, tag="m1")
            nc.vector.tensor_sub(m1, s2m, probsC)
            mst = smallp.tile([128, 16], f32, tag="mst")
            nc.vector.tensor_add(mst, m1, probsC)
            onem = smallp.tile([128, 16], f32, tag="onem")
            nc.vector.tensor_scalar(onem, mst, -1.0, 1.0, op0=Alu.mult, op1=Alu.add)

            # --- phase S: s1 matmul (f32r) + blend ---------------------------
            ws1r3 = ws1d[l].rearrange("(k p) e -> p k e", p=128)
            for ec in range(NEC):
                ws1 = ws1p.tile([128, TK, EC], f32r, tag="ws1")
                if WS1_MODE == "cast_dma":
                    nc.gpsimd.dma_start(
                        ws1, ws1r3[:, :, ec * EC : (ec + 1) * EC]
                    )
                else:
                    for hf in range(2):
                        e0 = ec * EC + hf * (EC // 2)
                        ws1raw = ws1rawp.tile(
                            [128, TK, EC // 2], f32, tag="ws1raw"
                        )
                        nc.sync.dma_start(
                            ws1raw, ws1r3[:, :, e0 : e0 + EC // 2]
                        )
                        nc.vector.tensor_copy(
                            ws1[:, :, hf * (EC // 2) : (hf + 1) * (EC // 2)],
                            ws1raw,
                        )
                biast = blendp.tile([128, EC], f32, tag="bias")
                nc.sync.dma_start(
                    biast,
                    bs1d[l, ec * EC : (ec + 1) * EC]
                    .unsqueeze(0)
                    .to_broadcast((128, EC)),
                )
                for t in range(TT):
                    ps = psSp.tile([128, EC], f32, tag="psS")
                    for k in range(TK):
                        nc.tensor.matmul(
                            ps,
                            hT[:, k, t * 128 : (t + 1) * 128],
                            ws1[:, k, :],
                            start=(k == 0),
                            stop=(k == TK - 1),
                        )
                    s2t = blendp.tile([128, EC], f32, tag="s2t")
                    nc.sync.dma_start(
                        s2t,
                        s2d[l, t * 128 : (t + 1) * 128, ec * EC : (ec + 1) * EC],
                    )
                    tmp = blendp.tile([128, EC], f32, tag="tmp")
                    nc.vector.tensor_scalar(
                        tmp, s2t, mst[:, t : t + 1], None, op0=Alu.mult
                    )
                    s1b = blendp.tile([128, EC], f32, tag="s1b")
                    nc.vector.tensor_add(s1b, ps, biast)
                    outt = blendp.tile([128, EC], f32, tag="s2t", name="outt")
                    nc.vector.scalar_tensor_tensor(
                        outt,
                        in0=s1b,
                        scalar=onem[:, t : t + 1],
                        in1=tmp,
                        op0=Alu.mult,
                        op1=Alu.add,
                    )
                    nc.scalar.dma_start(
                        outd[l, t * 128 : (t + 1) * 128, ec * EC : (ec + 1) * EC],
                        outt,
                    )

    nc.compile()
    _built = nc
    return _built


def kernel(**inputs):
    nc = _build()
    from concourse.bass_utils import run_bass_kernel_spmd

    hid = np.ascontiguousarray(
        np.asarray(inputs["hidden"], dtype=np.float32).reshape(L_TOT, T, D)
    )
    s2 = np.ascontiguousarray(
        np.asarray(inputs["s2"], dtype=np.float32).reshape(L_TOT, T, D)
    )
    wr1 = np.asarray(inputs["W_r1"], dtype=np.float32)
    br1 = np.asarray(inputs["b_r1"], dtype=np.float32)
    wr2 = np.asarray(inputs["W_r2"], dtype=np.float32)
    br2 = np.asarray(inputs["b_r2"], dtype=np.float32)
    ws1 = np.asarray(inputs["W_s1"], dtype=np.float32)
    bs1 = np.asarray(inputs["b_s1"], dtype=np.float32)

    in_maps = []
    for c in range(N_CORES):
        sl = slice(c * L_PER, (c + 1) * L_PER)
        in_maps.append(
            {
                "hidden": np.ascontiguousarray(hid[sl]),
                "s2": np.ascontiguousarray(s2[sl]),
                "w_r1": np.ascontiguousarray(wr1[sl]),
                "b_r1": np.ascontiguousarray(br1[sl]),
                "w_r2": np.ascontiguousarray(wr2[sl]),
                "b_r2": np.ascontiguousarray(br2[sl]),
                "w_s1": np.ascontiguousarray(ws1[sl]),
                "b_s1": np.ascontiguousarray(bs1[sl]),
            }
        )

    res = run_bass_kernel_spmd(nc, in_maps, core_ids=list(range(N_CORES)))
    out = np.concatenate(
        [r["out"].reshape(L_PER, B, T, D) for r in res.results], axis=0
    )
    return np.ascontiguousarray(out.astype(np.float32))


if __name__ == "__main__":
    _build()
    print("build OK")


# revision 16
# speedup vs baseline: 68596.6096x; 1.3085x over previous
"""Trainium2 Bass kernel for nn_MoCWrapper (topk_masking).

Reference semantics per layer l (L=16, B=1, T=2048, D=2048, R=128, K=1024):
  h            = relu(hidden @ W_r1 + b_r1)            # (T, R)
  logits       = h @ W_r2 + b_r2                       # (T,)
  probs        = sigmoid(logits)
  s2_mask      = one-hot top-K(probs) over tokens      # (T,)
  s1           = hidden @ W_s1 + b_s1                  # (T, D)
  m            = (s2_mask - probs) + probs
  out          = m * s2 + (1 - m) * s1

Sharding: layer dim L=16 across 8 cores (2 layers/core), SPMD one NEFF.

Per-core schedule per layer:
  phase T+R (fused): stream hidden tiles, PE-transpose each 128x128 block to
    PSUM, copy it twice: (a) DVE -> small fp32 streaming buffer consumed
    immediately by the fp32 router matmul (top-k selection must reproduce the
    reference ordering; boundary gaps are ~1e-5 in prob space so reduced
    precision is NOT safe here), (b) ACT -> resident hT tile declared float32r
    (rounds; these are hT's only writers, which the walrus fp32r verifier
    requires). Router logits per 128-token chunk -> row scratch (DRAM) +
    column layout [128,16] via tiny PE transposes of the same values.
  phase M: rank_i = #{j: logit_j > logit_i} via DVE tensor_scalar(is_gt) with
    fused accumulate; mask = rank < K. Equals lax.top_k selection when the
    K-th value is unique (holds for this data; verified in test).
  phase S: s1 = hT.T @ Ws1 in float32r (full PE rate at free-dim >= 256),
    Ws1 streamed in 256-wide f32r chunks; blend fused on PSUM with DVE.
"""

import numpy as np

L_TOT, B, T, D, R = 16, 1, 2048, 2048, 128
KSEL = 1024
N_CORES = 8
L_PER = L_TOT // N_CORES  # 2

TK = D // 128  # 16 contraction tiles
TT = T // 128  # 16 token chunks
EC = 256       # e-chunk width for s1/blend
NEC = D // EC  # 8

# how Ws1 gets rounded to f32r: "cast_dma" (gpsimd DMA casts on load) or
# "dve" (load raw f32 halves, DVE-rounds into the f32r tile)
WS1_MODE = "cast_dma"

_built = None


def _build():
    global _built
    if _built is not None:
        return _built

    import concourse.mybir as mybir
    import concourse.tile as tile
    from concourse import bacc
    from concourse.masks import make_identity

    f32 = mybir.dt.float32
    f32r = mybir.dt.float32r
    Alu = mybir.AluOpType
    Act = mybir.ActivationFunctionType
    X = mybir.AxisListType.X

    nc = bacc.Bacc("TRN2", target_bir_lowering=False, debug=False)

    hidden = nc.dram_tensor("hidden", [L_PER, T, D], f32, kind="ExternalInput").ap()
    s2d = nc.dram_tensor("s2", [L_PER, T, D], f32, kind="ExternalInput").ap()
    wr1d = nc.dram_tensor("w_r1", [L_PER, D, R], f32, kind="ExternalInput").ap()
    br1d = nc.dram_tensor("b_r1", [L_PER, R], f32, kind="ExternalInput").ap()
    wr2d = nc.dram_tensor("w_r2", [L_PER, R], f32, kind="ExternalInput").ap()
    br2d = nc.dram_tensor("b_r2", [L_PER], f32, kind="ExternalInput").ap()
    ws1d = nc.dram_tensor("w_s1", [L_PER, D, D], f32, kind="ExternalInput").ap()
    bs1d = nc.dram_tensor("b_s1", [L_PER, D], f32, kind="ExternalInput").ap()
    outd = nc.dram_tensor("out", [L_PER, T, D], f32, kind="ExternalOutput").ap()

    from contextlib import ExitStack

    with ExitStack() as ctx:
        tc = ctx.enter_context(tile.TileContext(nc))
        constp = ctx.enter_context(tc.tile_pool(name="constp", bufs=1))
        hTp = ctx.enter_context(tc.tile_pool(name="hTp", bufs=1))
        stgp = ctx.enter_context(tc.tile_pool(name="stgp", bufs=2))
        strmp = ctx.enter_context(tc.tile_pool(name="strmp", bufs=5))
        wtsp = ctx.enter_context(tc.tile_pool(name="wtsp", bufs=1))
        rowp = ctx.enter_context(tc.tile_pool(name="rowp", bufs=2))
        lbp = ctx.enter_context(tc.tile_pool(name="lbp", bufs=1))
        ws1p = ctx.enter_context(tc.tile_pool(name="ws1p", bufs=2))
        blendp = ctx.enter_context(tc.tile_pool(name="blendp", bufs=2))
        smallp = ctx.enter_context(tc.tile_pool(name="smallp", bufs=1))
        dramp = ctx.enter_context(tc.tile_pool(name="dramp", bufs=1, space="DRAM"))
        psTp = ctx.enter_context(tc.tile_pool(name="psTp", bufs=2, space="PSUM"))
        psRp = ctx.enter_context(tc.tile_pool(name="psRp", bufs=1, space="PSUM"))
        psLCp = ctx.enter_context(tc.tile_pool(name="psLCp", bufs=1, space="PSUM"))
        psSp = ctx.enter_context(tc.tile_pool(name="psSp", bufs=4, space="PSUM"))
        ws1rawp = ws1p if WS1_MODE == "cast_dma" else ctx.enter_context(
            tc.tile_pool(name="ws1rawp", bufs=2)
        )

        identity = constp.tile([128, 128], f32)
        make_identity(nc, identity)

        for l in range(L_PER):
            # --- per-layer weight loads -------------------------------------
            wr1 = wtsp.tile([128, TK, R], f32, tag="wr1")
            nc.sync.dma_start(wr1, wr1d[l].rearrange("(k p) r -> p k r", p=128))
            wr2 = smallp.tile([128, 1], f32, tag="wr2")
            nc.sync.dma_start(wr2, wr2d[l].unsqueeze(1))
            br1 = smallp.tile([128, 1], f32, tag="br1")
            nc.sync.dma_start(br1, br1d[l].unsqueeze(1))
            br2c = smallp.tile([128, 1], f32, tag="br2c")
            nc.sync.dma_start(br2c, br2d[l : l + 1].unsqueeze(0).to_broadcast((128, 1)))

            # --- fused transpose + router over t-chunks ----------------------
            hT = hTp.tile([128, TK, T], f32r, tag="hT")
            scr = dramp.tile([1, T], f32, tag="scr")
            psLC = psLCp.tile([128, 16], f32, tag="psLC")
            for t in range(TT):
                psR = psRp.tile([128, 128], f32, tag="psR")
                strms = []
                for half in range(2):
                    stg = stgp.tile([128, 1024], f32, tag="stg")
                    nc.sync.dma_start(
                        stg,
                        hidden[l, t * 128 : (t + 1) * 128,
                               half * 1024 : (half + 1) * 1024],
                    )
                    for g in range(2):
                        psT = psTp.tile([128, 512], f32, tag="psT")
                        for i in range(4):
                            nc.tensor.transpose(
                                psT[:, i * 128 : (i + 1) * 128],
                                stg[:, (g * 4 + i) * 128 : (g * 4 + i + 1) * 128],
                                identity,
                            )
                        k0 = half * 8 + g * 4
                        psT3 = psT.rearrange("p (a b) -> p a b", a=4)
                        strm = strmp.tile([128, 4, 128], f32, tag="strm")
                        nc.vector.tensor_copy(strm, psT3)
                        nc.scalar.activation(
                            hT[:, k0 : k0 + 4, t * 128 : (t + 1) * 128],
                            psT3,
                            Act.Copy,
                        )
                        strms.append((k0, strm))
                # dense fp32 router matmul for this t-chunk (N=128)
                for k0, strm in strms:
                    for i in range(4):
                        nc.tensor.matmul(
                            psR,
                            wr1[:, k0 + i, :],
                            strm[:, i, :],
                            start=(k0 + i == 0),
                            stop=(k0 + i == TK - 1),
                        )
                hrelu = rowp.tile([128, 128], f32, tag="hrelu")
                nc.scalar.activation(hrelu, psR, Act.Relu, bias=br1)
                # logits directly in column layout: out[128t, 1] at column t
                nc.tensor.matmul(
                    psLC[:, t : t + 1], hrelu, wr2, start=True, stop=True
                )

            # row layout derives from the same psLC values (bit-exact copies):
            # colsb -> DRAM scratch (strided so scr[t] = logit_t) -> broadcast
            colsb = smallp.tile([128, 16], f32, tag="colsb")
            nc.scalar.activation(colsb, psLC, Act.Copy)
            nc.scalar.dma_start(
                scr[0:1, :].rearrange("a (c p) -> (a p) c", p=128), colsb
            )

            # --- phase M: rank -> mask -> blend coefficients -----------------
            lb = lbp.tile([128, T], f32, tag="lb")
            nc.sync.dma_start(lb, scr[0:1, :].to_broadcast((128, T)))
            rank4 = smallp.tile([128, 16, 4], f32, tag="rank4")
            for c in range(16):
                for q in range(4):
                    cmp = stgp.tile([128, 512], f32, tag="stg", name="cmp")
                    nc.vector.tensor_scalar(
                        cmp,
                        lb[:, q * 512 : (q + 1) * 512],
                        psLC[:, c : c + 1],
                        None,
                        op0=Alu.is_gt,
                        op1=Alu.add,
                        accum_out=rank4[:, c, q : q + 1],
                    )
            rank = smallp.tile([128, 16], f32, tag="rank")
            nc.vector.tensor_reduce(rank, rank4, axis=X, op=Alu.add)
            probsC = smallp.tile([128, 16], f32, tag="probsC")
            nc.scalar.activation(probsC, psLC, Act.Sigmoid, bias=br2c)
            s2m = smallp.tile([128, 16], f32, tag="s2m")
            nc.vector.tensor_scalar(s2m, rank, float(KSEL), None, op0=Alu.is_lt)
            m1 = smallp.tile([128, 16], f32, tag="m1")
            nc.vector.tensor_sub(m1, s2m, probsC)
            mst = smallp.tile([128, 16], f32, tag="mst")
            nc.vector.tensor_add(mst, m1, probsC)
            onem = smallp.tile([128, 16], f32, tag="onem")
            nc.vector.tensor_scalar(onem, mst, -1.0, 1.0, op0=Alu.mult, op1=Alu.add)

            # --- phase S: s1 matmul (f32r) + blend ---------------------------
            ws1r3 = ws1d[l].rearrange("(k p) e -> p k e", p=128)
            for ec in range(NEC):
                ws1 = ws1p.tile([128, TK, EC], f32r, tag="ws1")
                if WS1_MODE == "cast_dma":
                    nc.gpsimd.dma_start(
                        ws1, ws1r3[:, :, ec * EC : (ec + 1) * EC]
                    )
                else:
                    for hf in range(2):
                        e0 = ec * EC + hf * (EC // 2)
                        ws1raw = ws1rawp.tile(
                            [128, TK, EC // 2], f32, tag="ws1raw"
                        )
                        nc.sync.dma_start(
                            ws1raw, ws1r3[:, :, e0 : e0 + EC // 2]
                        )
                        nc.vector.tensor_copy(
                            ws1[:, :, hf * (EC // 2) : (hf + 1) * (EC // 2)],
                            ws1raw,
                        )
                biast = blendp.tile([128, EC], f32, tag="bias")
                nc.sync.dma_start(
                    biast,
                    bs1d[l, ec * EC : (ec + 1) * EC]
                    .unsqueeze(0)
                    .to_broadcast((128, EC)),
                )
                for t in range(TT):
                    ps = psSp.tile([128, EC], f32, tag="psS")
                    for k in range(TK):
                        nc.tensor.matmul(
                            ps,
                            hT[:, k, t * 128 : (t + 1) * 128],
                            ws1[:, k, :],
                            start=(k == 0),
                            stop=(k == TK - 1),
                        )
                    s2t = blendp.tile([128, EC], f32, tag="s2t", bufs=4)
                    nc.sync.dma_start(
                        s2t,
                        s2d[l, t * 128 : (t + 1) * 128, ec * EC : (ec + 1) * EC],
                    )
                    tmp = blendp.tile([128, EC], f32, tag="tmp")
                    nc.vector.tensor_scalar(
                        tmp, s2t, mst[:, t : t + 1], None, op0=Alu.mult
                    )
                    s1b = blendp.tile([128, EC], f32, tag="s1b")
                    nc.vector.tensor_add(s1b, ps, biast)
                    outt = blendp.tile([128, EC], f32, tag="s2t", bufs=4, name="outt")
                    nc.vector.scalar_tensor_tensor(
                        outt,
                        in0=s1b,
                        scalar=onem[:, t : t + 1],
                        in1=tmp,
                        op0=Alu.mult,
                        op1=Alu.add,
                    )
                    nc.scalar.dma_start(
                        outd[l, t * 128 : (t + 1) * 128, ec * EC : (ec + 1) * EC],
                        outt,
                    )

    nc.compile()
    _built = nc
    return _built


def kernel(**inputs):
    nc = _build()
    from concourse.bass_utils import run_bass_kernel_spmd

    hid = np.ascontiguousarray(
        np.asarray(inputs["hidden"], dtype=np.float32).reshape(L_TOT, T, D)
    )
    s2 = np.ascontiguousarray(
        np.asarray(inputs["s2"], dtype=np.float32).reshape(L_TOT, T, D)
    )
    wr1 = np.asarray(inputs["W_r1"], dtype=np.float32)
    br1 = np.asarray(inputs["b_r1"], dtype=np.float32)
    wr2 = np.asarray(inputs["W_r2"], dtype=np.float32)
    br2 = np.asarray(inputs["b_r2"], dtype=np.float32)
    ws1 = np.asarray(inputs["W_s1"], dtype=np.float32)
    bs1 = np.asarray(inputs["b_s1"], dtype=np.float32)

    in_maps = []
    for c in range(N_CORES):
        sl = slice(c * L_PER, (c + 1) * L_PER)
        in_maps.append(
            {
                "hidden": np.ascontiguousarray(hid[sl]),
                "s2": np.ascontiguousarray(s2[sl]),
                "w_r1": np.ascontiguousarray(wr1[sl]),
                "b_r1": np.ascontiguousarray(br1[sl]),
                "w_r2": np.ascontiguousarray(wr2[sl]),
                "b_r2": np.ascontiguousarray(br2[sl]),
                "w_s1": np.ascontiguousarray(ws1[sl]),
                "b_s1": np.ascontiguousarray(bs1[sl]),
            }
        )

    res = run_bass_kernel_spmd(nc, in_maps, core_ids=list(range(N_CORES)))
    out = np.concatenate(
        [r["out"].reshape(L_PER, B, T, D) for r in res.results], axis=0
    )
    return np.ascontiguousarray(out.astype(np.float32))


if __name__ == "__main__":
    _build()
    print("build OK")
